# revision 7
# baseline (speedup 1.0000x reference)
"""AdaptiveSampler Trainium2 kernel: batch-parallel frame gather across 8 NeuronCores.

Reference semantics: out[b, j*4+g] = x[b, ceil(mu[b,j,g])] (zero frame when the
sampled index falls outside [0, T-1]), with
  mu[b,j,g] = (dt[b,j]*31.5 + 31.5) + (g - 1.5) * ((64/3 - 1)*delta_t[b,j] + 1).

Strategy: pure data parallelism over batch (4 samples/core). The sampled frame
indices are computed host-side (bit-identical to the jax reference, on jax-CPU)
and shipped as a tiny int32 tensor. On-device the kernel is a pipelined
indirect-DMA gather (HBM->SBUF) + indirect scatter (SBUF->HBM); out-of-range
anchors are skipped on both sides, so zero frames come from the pre-zeroed
output buffer and cost no HBM traffic.

Frames are split into SUB subrows because the indirect-DMA row size is a 16-bit
byte field (<= 65535 B); a full 150528 B frame doesn't fit, a 37632 B quarter
does.
"""

import numpy as np

import concourse.bass as bass
import concourse.mybir as mybir
from concourse.bass_utils import run_bass_kernel_spmd

B, T, C, H, W = 32, 64, 3, 112, 112
AOT = 4                      # output frames per anchor; 4 anchors
NCORES = 8
BL = B // NCORES             # local batches per core
CHW = C * H * W              # 37632 floats per frame
SUB = 4                      # subrows per frame (row bytes must be < 64 KiB)
SUBLEN = CHW // SUB          # 9408 floats = 37632 B per subrow
NROWS_IN = BL * T * SUB      # 1024 source subrows per core
FRAMES_OUT = BL * AOT * AOT  # 64 output frames per core
NROWS_OUT = FRAMES_OUT * SUB # 256 output subrows per core
NPART = 128
NBLK = NROWS_OUT // NPART    # 2 column blocks in SBUF
NCHUNK = 2                   # pipeline chunks; chunks must be full-128-partition DMAs
                             # (partition-offset slices in indirect DMA fail at runtime)
LOOKAHEAD = 2                # gathers issued ahead of scatters
OOB = 1 << 30

TRACE = False
RUN_KWARGS = {}
LAST_RESULT = None

_graph_cache = {}


def _build_graph():
    nc = bass.Bass()
    xz = nc.declare_dram_parameter("xz", [NROWS_IN, SUBLEN], mybir.dt.float32, isOutput=False)
    idx = nc.declare_dram_parameter("idx", [NPART, 2 * NBLK], mybir.dt.int32, isOutput=False)
    out = nc.declare_dram_parameter("out", [NROWS_OUT, SUBLEN], mybir.dt.float32, isOutput=True)

    rows_per_chunk = NROWS_OUT // NCHUNK          # subrows per chunk
    parts_per_chunk = rows_per_chunk              # one subrow per partition within a block
    assert NPART % parts_per_chunk == 0

    with (
        nc.sbuf_tensor("buf", [NPART, NBLK * SUBLEN], mybir.dt.float32) as buf,
        nc.sbuf_tensor("idxs", [NPART, 2 * NBLK], mybir.dt.int32) as idxs,
        nc.semaphore("s_idx") as s_idx,
        nc.semaphore("s_g") as s_g,
        nc.semaphore("s_s") as s_s,
        nc.Block() as block,
    ):
        @block.sync
        def _(sync):
            sync.dma_start(out=idxs[:, :], in_=idx[:, :]).then_inc(s_idx, 16)

        @block.gpsimd
        def _(gpsimd):
            gpsimd.wait_ge(s_idx, 16)

            def chunk_slices(k):
                s0 = k * rows_per_chunk                 # first global subrow of chunk
                g = s0 // NPART                         # column block
                p0 = s0 % NPART                         # first partition
                rows = slice(p0, p0 + parts_per_chunk)
                cols = slice(g * SUBLEN, (g + 1) * SUBLEN)
                return rows, cols, g

            def gather(k):
                rows, cols, g = chunk_slices(k)
                gpsimd.indirect_dma_start(
                    out=buf[rows, cols],
                    out_offset=None,
                    in_=xz[:, :],
                    in_offset=bass.IndirectOffsetOnAxis(ap=idxs[rows, 2 * g:2 * g + 1], axis=0),
                    bounds_check=NROWS_IN - 1,
                    oob_is_err=False,
                ).then_inc(s_g, 16)

            def scatter(k):
                rows, cols, g = chunk_slices(k)
                gpsimd.indirect_dma_start(
                    out=out[:, :],
                    out_offset=bass.IndirectOffsetOnAxis(ap=idxs[rows, 2 * g + 1:2 * g + 2], axis=0),
                    in_=buf[rows, cols],
                    in_offset=None,
                    bounds_check=NROWS_OUT - 1,
                    oob_is_err=False,
                ).then_inc(s_s, 16)

            issued = 0
            for c in range(NCHUNK):
                while issued < min(c + LOOKAHEAD, NCHUNK):
                    gather(issued)
                    issued += 1
                gpsimd.wait_ge(s_g, 16 * (c + 1))
                scatter(c)
            gpsimd.wait_ge(s_s, 16 * NCHUNK)

    return nc


def _get_graph():
    if "nc" not in _graph_cache:
        _graph_cache["nc"] = _build_graph()
    return _graph_cache["nc"]


def _frame_indices(dt, delta_t):
    """ceil(mu) per (b, j, g), bit-identical to the jax reference (on jax-CPU)."""
    import jax
    import jax.numpy as jnp

    with jax.default_device(jax.devices("cpu")[0]):
        dtj = jnp.asarray(np.asarray(dt, dtype=np.float32))
        dlj = jnp.asarray(np.asarray(delta_t, dtype=np.float32))
        anchor_t = (T - 1) / 2.0
        dts = dtj * anchor_t + anchor_t
        deltas = (T / (AOT - 1) - 1.0) * dlj + 1.0
        grid = jnp.arange(AOT, dtype=jnp.float32)
        mu = dts[:, :, None] + (grid[None, None, :] - (AOT - 1) / 2.0) * deltas[:, :, None]
        idxf = np.asarray(jnp.ceil(mu))  # [B, AOT, AOT] float32
    valid = (idxf >= 0) & (idxf <= T - 1)
    t_idx = np.where(valid, idxf, 0).astype(np.int64)
    return t_idx.reshape(B, AOT * AOT), valid.reshape(B, AOT * AOT)


def kernel(x, dt, delta_t):
    global LAST_RESULT
    x = np.ascontiguousarray(np.asarray(x), dtype=np.float32)
    t_flat, v_flat = _frame_indices(dt, delta_t)

    # global output subrow s (0..255): frame q = s // SUB, subrow sub = s % SUB
    s = np.arange(NROWS_OUT)
    q = s // SUB
    sub = s % SUB
    bl = q // (AOT * AOT)
    f = q % (AOT * AOT)

    # Descriptors of one indirect DMA are striped over SDMA engines in
    # partition-QUADS: engine = ((p//4) + 8*dma_index) % 16 (measured). OOB
    # subrows are skipped at the descriptor level, so balance the VALID
    # subrows evenly across the 32 quads of each column-block DMA so every
    # engine moves the same number of descriptors. Unused slots get OOB.
    in_maps = []
    for m in range(NCORES):
        xs = x[m * BL:(m + 1) * BL].reshape(NROWS_IN, SUBLEN)
        b = m * BL + bl
        src_all = SUB * (bl * T + t_flat[b, f]) + sub
        ok = v_flat[b, f]
        vs = s[ok]                        # valid subrows, any order
        i = np.arange(len(vs))
        blk = i % NBLK
        k = i // NBLK                     # rank within block (0..127)
        part = (k % 32) * 4 + k // 32
        idx_np = np.full((NPART, 2 * NBLK), OOB, np.int32)
        idx_np[part, 2 * blk] = src_all[vs]
        idx_np[part, 2 * blk + 1] = vs
        in_maps.append({"xz": xs, "idx": idx_np})

    nc = _get_graph()
    LAST_RESULT = run_bass_kernel_spmd(
        nc, in_maps, core_ids=list(range(NCORES)), trace=TRACE, **RUN_KWARGS
    )
    outs = [r["out"].reshape(BL, AOT * AOT, C, H, W) for r in LAST_RESULT.results]
    return np.concatenate(outs, axis=0)


# revision 8
# speedup vs baseline: 1.1185x; 1.1185x over previous
"""AdaptiveSampler Trainium2 kernel: batch-parallel frame gather across 8 NeuronCores.

Reference semantics: out[b, j*4+g] = x[b, ceil(mu[b,j,g])] (zero frame when the
sampled index falls outside [0, T-1]), with
  mu[b,j,g] = (dt[b,j]*31.5 + 31.5) + (g - 1.5) * ((64/3 - 1)*delta_t[b,j] + 1).

Strategy: pure data parallelism over batch (4 samples/core). The sampled frame
indices are computed host-side (bit-identical to the jax reference, on jax-CPU)
and shipped as a tiny int32 tensor. On-device the kernel is a pipelined
indirect-DMA gather (HBM->SBUF) + indirect scatter (SBUF->HBM); out-of-range
anchors are skipped on both sides, so zero frames come from the pre-zeroed
output buffer and cost no HBM traffic.

Frames are split into SUB subrows because the indirect-DMA row size is a 16-bit
byte field (<= 65535 B); a full 150528 B frame doesn't fit, a 37632 B quarter
does.
"""

import numpy as np

import concourse.bass as bass
import concourse.mybir as mybir
from concourse.bass_utils import run_bass_kernel_spmd

B, T, C, H, W = 32, 64, 3, 112, 112
AOT = 4                      # output frames per anchor; 4 anchors
NCORES = 8
BL = B // NCORES             # local batches per core
CHW = C * H * W              # 37632 floats per frame
SUB = 4                      # subrows per frame (row bytes must be < 64 KiB)
SUBLEN = CHW // SUB          # 9408 floats = 37632 B per subrow
NROWS_IN = BL * T * SUB      # 1024 source subrows per core
FRAMES_OUT = BL * AOT * AOT  # 64 output frames per core
NROWS_OUT = FRAMES_OUT * SUB # 256 output subrows per core
NPART = 128
NBLK = NROWS_OUT // NPART    # 2 column blocks in SBUF
NCHUNK = 2                   # pipeline chunks; chunks must be full-128-partition DMAs
                             # (partition-offset slices in indirect DMA fail at runtime)
LOOKAHEAD = 2                # gathers issued ahead of scatters
OOB = 1 << 30

TRACE = False
RUN_KWARGS = {}
LAST_RESULT = None

_graph_cache = {}


def _build_graph():
    nc = bass.Bass()
    xz = nc.declare_dram_parameter("xz", [NROWS_IN, SUBLEN], mybir.dt.float32, isOutput=False)
    idx = nc.declare_dram_parameter("idx", [NPART, 2 * NBLK], mybir.dt.int32, isOutput=False)
    out = nc.declare_dram_parameter("out", [NROWS_OUT, SUBLEN], mybir.dt.float32, isOutput=True)

    rows_per_chunk = NROWS_OUT // NCHUNK          # subrows per chunk
    parts_per_chunk = rows_per_chunk              # one subrow per partition within a block
    assert NPART % parts_per_chunk == 0

    with (
        nc.sbuf_tensor("buf", [NPART, NBLK * SUBLEN], mybir.dt.float32) as buf,
        nc.sbuf_tensor("idxs", [NPART, 2 * NBLK], mybir.dt.int32) as idxs,
        nc.semaphore("s_idx") as s_idx,
        nc.semaphore("s_g") as s_g,
        nc.semaphore("s_s") as s_s,
        nc.Block() as block,
    ):
        @block.sync
        def _(sync):
            sync.dma_start(out=idxs[:, :], in_=idx[:, :]).then_inc(s_idx, 16)

        @block.gpsimd
        def _(gpsimd):
            gpsimd.wait_ge(s_idx, 16)

            def chunk_slices(k):
                s0 = k * rows_per_chunk                 # first global subrow of chunk
                g = s0 // NPART                         # column block
                p0 = s0 % NPART                         # first partition
                rows = slice(p0, p0 + parts_per_chunk)
                cols = slice(g * SUBLEN, (g + 1) * SUBLEN)
                return rows, cols, g

            def gather(k):
                rows, cols, g = chunk_slices(k)
                gpsimd.indirect_dma_start(
                    out=buf[rows, cols],
                    out_offset=None,
                    in_=xz[:, :],
                    in_offset=bass.IndirectOffsetOnAxis(ap=idxs[rows, 2 * g:2 * g + 1], axis=0),
                    bounds_check=NROWS_IN - 1,
                    oob_is_err=False,
                ).then_inc(s_g, 16)

            def scatter(k):
                rows, cols, g = chunk_slices(k)
                gpsimd.indirect_dma_start(
                    out=out[:, :],
                    out_offset=bass.IndirectOffsetOnAxis(ap=idxs[rows, 2 * g + 1:2 * g + 2], axis=0),
                    in_=buf[rows, cols],
                    in_offset=None,
                    bounds_check=NROWS_OUT - 1,
                    oob_is_err=False,
                ).then_inc(s_s, 16)

            issued = 0
            for c in range(NCHUNK):
                while issued < min(c + LOOKAHEAD, NCHUNK):
                    gather(issued)
                    issued += 1
                gpsimd.wait_ge(s_g, 16 * (c + 1))
                scatter(c)
            gpsimd.wait_ge(s_s, 16 * NCHUNK)

    return nc


def _get_graph():
    if "nc" not in _graph_cache:
        _graph_cache["nc"] = _build_graph()
    return _graph_cache["nc"]


def _frame_indices(dt, delta_t):
    """ceil(mu) per (b, j, g), bit-identical to the jax reference (on jax-CPU)."""
    import jax
    import jax.numpy as jnp

    with jax.default_device(jax.devices("cpu")[0]):
        dtj = jnp.asarray(np.asarray(dt, dtype=np.float32))
        dlj = jnp.asarray(np.asarray(delta_t, dtype=np.float32))
        anchor_t = (T - 1) / 2.0
        dts = dtj * anchor_t + anchor_t
        deltas = (T / (AOT - 1) - 1.0) * dlj + 1.0
        grid = jnp.arange(AOT, dtype=jnp.float32)
        mu = dts[:, :, None] + (grid[None, None, :] - (AOT - 1) / 2.0) * deltas[:, :, None]
        idxf = np.asarray(jnp.ceil(mu))  # [B, AOT, AOT] float32
    valid = (idxf >= 0) & (idxf <= T - 1)
    t_idx = np.where(valid, idxf, 0).astype(np.int64)
    return t_idx.reshape(B, AOT * AOT), valid.reshape(B, AOT * AOT)


def kernel(x, dt, delta_t):
    global LAST_RESULT
    x = np.ascontiguousarray(np.asarray(x), dtype=np.float32)
    t_flat, v_flat = _frame_indices(dt, delta_t)

    # global output subrow s (0..255): frame q = s // SUB, subrow sub = s % SUB
    s = np.arange(NROWS_OUT)
    q = s // SUB
    sub = s % SUB
    bl = q // (AOT * AOT)
    f = q % (AOT * AOT)

    # Indirect-DMA descriptors map to SDMA engines by SBUF partition:
    # engine(p) = ((p//4) % 8)*2 + p//64 (measured empirically). OOB subrows
    # are skipped at the descriptor level (4-byte dummy), so balance the VALID
    # subrows round-robin across the 16 engines (and across the 2 column-block
    # DMAs within an engine) so every engine moves the same number of
    # descriptors. Unused slots get OOB.
    eng = np.arange(NROWS_OUT) % 16
    rank = np.arange(NROWS_OUT) // 16     # slot rank within engine (0..15)
    blk_a = rank % NBLK
    jj = rank // NBLK                     # 0..7: which of the engine's 8 partitions
    part_a = 4 * (eng // 2) + 64 * (eng % 2) + 32 * (jj // 4) + jj % 4

    in_maps = []
    for m in range(NCORES):
        xs = x[m * BL:(m + 1) * BL].reshape(NROWS_IN, SUBLEN)
        b = m * BL + bl
        src_all = SUB * (bl * T + t_flat[b, f]) + sub
        ok = v_flat[b, f]
        vs = s[ok]                        # valid subrows, any order
        n = len(vs)
        idx_np = np.full((NPART, 2 * NBLK), OOB, np.int32)
        idx_np[part_a[:n], 2 * blk_a[:n]] = src_all[vs]
        idx_np[part_a[:n], 2 * blk_a[:n] + 1] = vs
        in_maps.append({"xz": xs, "idx": idx_np})

    nc = _get_graph()
    LAST_RESULT = run_bass_kernel_spmd(
        nc, in_maps, core_ids=list(range(NCORES)), trace=TRACE, **RUN_KWARGS
    )
    outs = [r["out"].reshape(BL, AOT * AOT, C, H, W) for r in LAST_RESULT.results]
    return np.concatenate(outs, axis=0)
